# revision 22
# baseline (speedup 1.0000x reference)
"""Trainium2 Bass kernel for the AttnProcessor+LoRA capture problem.

Reference (per batch element):
    q = hs @ Wq.T + s*(hs @ Aq.T) @ Bq.T
    k = enc @ Wk.T + s*(enc @ Ak.T) @ Bk.T
    v = enc @ Wv.T + s*(enc @ Av.T) @ Bv.T
    score[h] = (q_h @ k_h.T) / 8 ;  prob = softmax(score, -1)
    out = prob @ v ;  y = out @ Wo.T + s*(out @ Ao.T) @ Bo.T + bo
Returns (y, score, prob).

Distribution: pure data-parallel over batch B=8 -> one element per
NeuronCore, no collectives.  Matmuls run bf16 with fp32 PSUM accumulation;
softmax in fp32.  Host-side layout prep: activations fed feature-major
([feat, tokens]) so contractions land on the partition axis; score/prob
produced as [H, S, L] on device (2KB-row DMAs) and permuted on host; the
1/sqrt(64) and LoRA scales are folded into the weights.  Softmax skips
max-subtraction (|score| < ~4) and normalizes via a gpsimd cross-partition
all-reduce of exp over S=77.
"""

import numpy as np
import ml_dtypes

import concourse.bass as bass
import concourse.tile as tile
from concourse import bacc, mybir
from concourse.bass_utils import run_bass_kernel_spmd

HEADS = 20
HEAD_DIM = 64
INNER = 1280
CTX = 768
LORA_R = 128
LORA_SCALE = 16.0 / 128.0
B, L, S = 8, 4096, 77
SCORE_SCALE = 1.0 / 8.0  # 1/sqrt(64)

LC = 256              # tokens per pipeline chunk
NLC = L // LC         # 16
KQ = INNER // 128     # 10
KE = CTX // 128       # 6
FQ = INNER // 128     # 10
GH = 4                # heads per attention group
NG = HEADS // GH      # 5
FO_CHUNKS = [(0, 512), (512, 512), (1024, 256)]

BF16 = mybir.dt.bfloat16
F32 = mybir.dt.float32

_COMPILED = None
TRACE = False          # set True (with axon_profile imported) to capture NTFF
LAST_RESULT = None     # BassKernelResults of the most recent kernel() call


def _build_program():
    nc = bacc.Bacc("TRN2", target_bir_lowering=False, debug=False)

    d_xT = nc.declare_dram_parameter("xT", [KQ, 128, L], BF16, isOutput=False)
    d_encT = nc.declare_dram_parameter("encT", [KE, 128, S], BF16, isOutput=False)
    d_wqT = nc.declare_dram_parameter("wqT", [KQ, 128, INNER], BF16, isOutput=False)
    d_wkT = nc.declare_dram_parameter("wkT", [KE, 128, INNER], BF16, isOutput=False)
    d_wvT = nc.declare_dram_parameter("wvT", [KE, 128, INNER], BF16, isOutput=False)
    d_woT = nc.declare_dram_parameter("woT", [KQ, 128, INNER], BF16, isOutput=False)
    d_aqT = nc.declare_dram_parameter("aqT", [KQ, 128, LORA_R], BF16, isOutput=False)
    d_akT = nc.declare_dram_parameter("akT", [KE, 128, LORA_R], BF16, isOutput=False)
    d_avT = nc.declare_dram_parameter("avT", [KE, 128, LORA_R], BF16, isOutput=False)
    d_aoT = nc.declare_dram_parameter("aoT", [KQ, 128, LORA_R], BF16, isOutput=False)
    d_bqT = nc.declare_dram_parameter("bqT", [LORA_R, INNER], BF16, isOutput=False)
    d_bkT = nc.declare_dram_parameter("bkT", [LORA_R, INNER], BF16, isOutput=False)
    d_bvT = nc.declare_dram_parameter("bvT", [LORA_R, INNER], BF16, isOutput=False)
    d_boT = nc.declare_dram_parameter("boT", [LORA_R, INNER], BF16, isOutput=False)
    d_bias = nc.declare_dram_parameter("bias", [INNER], F32, isOutput=False)

    d_y = nc.declare_dram_parameter("y", [L, INNER], F32, isOutput=True)
    d_scoreT = nc.declare_dram_parameter("scoreT", [HEADS, S, L], F32, isOutput=True)
    d_probT = nc.declare_dram_parameter("probT", [HEADS, S, L], F32, isOutput=True)

    with tile.TileContext(nc) as tc:
        with (
            tc.tile_pool(name="wts", bufs=1) as wts,
            tc.tile_pool(name="kv", bufs=1) as kvp,
        ):
            # ---- persistent weights ----
            w_q = wts.tile([128, KQ, INNER], BF16)
            w_o = wts.tile([128, KQ, INNER], BF16)
            a_q = wts.tile([128, KQ, LORA_R], BF16)
            a_o = wts.tile([128, KQ, LORA_R], BF16)
            b_q = wts.tile([128, INNER], BF16)
            b_o = wts.tile([128, INNER], BF16)
            bias_b = wts.tile([128, INNER], F32)
            for dst, src in ((w_q, d_wqT), (w_o, d_woT), (a_q, d_aqT), (a_o, d_aoT)):
                nc.sync.dma_start(out=dst, in_=src.ap().transpose([1, 0, 2]))
            nc.sync.dma_start(out=b_q, in_=d_bqT.ap())
            nc.sync.dma_start(out=b_o, in_=d_boT.ap())
            bias_ap = d_bias.ap()
            nc.gpsimd.dma_start(
                out=bias_b,
                in_=bass.AP(tensor=bias_ap.tensor, offset=bias_ap.offset,
                            ap=[[0, 128]] + list(bias_ap.ap)),
            )

            # k^T [feat, S] and v [S, heads, dim], computed once
            kt = kvp.tile([128, FQ, S], BF16)
            v_sb = kvp.tile([S, HEADS, HEAD_DIM], BF16)

            with (
                tc.tile_pool(name="setup", bufs=1) as stp,
                tc.tile_pool(name="ps_setup", bufs=2, space="PSUM") as pss,
            ):
                w_k = stp.tile([128, KE, INNER], BF16)
                w_v = stp.tile([128, KE, INNER], BF16)
                a_k = stp.tile([128, KE, LORA_R], BF16)
                a_v = stp.tile([128, KE, LORA_R], BF16)
                b_k = stp.tile([128, INNER], BF16)
                b_v = stp.tile([128, INNER], BF16)
                enc_sb = stp.tile([128, KE, S], BF16)
                for dst, src in ((w_k, d_wkT), (w_v, d_wvT),
                                 (a_k, d_akT), (a_v, d_avT)):
                    nc.sync.dma_start(out=dst, in_=src.ap().transpose([1, 0, 2]))
                nc.sync.dma_start(out=b_k, in_=d_bkT.ap())
                nc.sync.dma_start(out=b_v, in_=d_bvT.ap())
                nc.sync.dma_start(out=enc_sb, in_=d_encT.ap().transpose([1, 0, 2]))

                tk_ps = pss.tile([128, S], F32, tag="ps_lora")
                for c in range(KE):
                    nc.tensor.matmul(tk_ps, a_k[:, c, :], enc_sb[:, c, :],
                                     start=(c == 0), stop=(c == KE - 1))
                tk_sb = stp.tile([128, S], BF16)
                nc.scalar.copy(tk_sb, tk_ps)
                tv_ps = pss.tile([128, S], F32, tag="ps_lora")
                for c in range(KE):
                    nc.tensor.matmul(tv_ps, a_v[:, c, :], enc_sb[:, c, :],
                                     start=(c == 0), stop=(c == KE - 1))
                tv_sb = stp.tile([128, S], BF16)
                nc.scalar.copy(tv_sb, tv_ps)

                for f in range(FQ):
                    ps_k = pss.tile([128, S], F32, tag="ps_k")
                    for c in range(KE):
                        nc.tensor.matmul(ps_k, w_k[:, c, bass.ts(f, 128)],
                                         enc_sb[:, c, :], start=(c == 0), stop=False)
                    nc.tensor.matmul(ps_k, b_k[:, bass.ts(f, 128)], tk_sb,
                                     start=False, stop=True)
                    nc.scalar.copy(kt[:, f, :], ps_k)

                for f0, fn in FO_CHUNKS:
                    ps_v = pss.tile([S, 512], F32, tag="ps_v")
                    for c in range(KE):
                        nc.tensor.matmul(ps_v[:, :fn], enc_sb[:, c, :],
                                         w_v[:, c, f0:f0 + fn],
                                         start=(c == 0), stop=False)
                    nc.tensor.matmul(ps_v[:, :fn], tv_sb, b_v[:, f0:f0 + fn],
                                     start=False, stop=True)
                    nc.vector.tensor_copy(
                        v_sb[:, f0 // HEAD_DIM:(f0 + fn) // HEAD_DIM, :],
                        ps_v[:, :fn].rearrange("s (h d) -> s h d", d=HEAD_DIM),
                    )

            # ---- main pipeline over L-chunks ----
            with (
                tc.tile_pool(name="acts", bufs=2) as acts,
                tc.tile_pool(name="attn", bufs=3) as attn,
                tc.tile_pool(name="attnL", bufs=NG + 2) as attnL,
                tc.tile_pool(name="outs", bufs=2) as outs,
                tc.tile_pool(name="ps_mm", bufs=2, space="PSUM") as ps_mm,
                tc.tile_pool(name="ps_at", bufs=4, space="PSUM") as ps_at,
                tc.tile_pool(name="dram", bufs=NG + 1, space="DRAM") as dramp,
            ):
                # indicator matrices: eg[:, j, :] is [S, GH] with column j
                # all-ones -> matmul accumulates head j's column-sums into
                # row j of a [GH, LC] PSUM tile
                eg_f = wts.tile([S, GH, GH], F32)
                nc.vector.memset(eg_f, 0.0)
                for j in range(GH):
                    nc.vector.memset(eg_f[:, j, j:j + 1], 1.0)
                eg = wts.tile([S, GH, GH], mybir.dt.float32r)
                nc.vector.tensor_copy(eg, eg_f)
                def emit_qproj(lc):
                    sl = bass.ts(lc, LC)
                    xq = acts.tile([128, KQ, LC], BF16, tag="xq")
                    nc.sync.dma_start(out=xq,
                                      in_=d_xT.ap()[:, :, sl].transpose([1, 0, 2]))
                    tq_ps = ps_mm.tile([128, LC], F32, tag="ps_q")
                    for c in range(KQ):
                        nc.tensor.matmul(tq_ps, a_q[:, c, :], xq[:, c, :],
                                         start=(c == 0), stop=(c == KQ - 1))
                    tq = acts.tile([128, LC], BF16, tag="tq")
                    nc.scalar.copy(tq, tq_ps)
                    qt = acts.tile([128, FQ, LC], BF16, tag="qt")
                    for f in range(FQ):
                        ps_q = ps_mm.tile([128, LC], F32, tag="ps_q")
                        for c in range(KQ):
                            nc.tensor.matmul(ps_q, w_q[:, c, bass.ts(f, 128)],
                                             xq[:, c, :], start=(c == 0), stop=False)
                        nc.tensor.matmul(ps_q, b_q[:, bass.ts(f, 128)], tq,
                                         start=False, stop=True)
                        nc.scalar.copy(qt[:, f, :], ps_q)
                    return qt

                def emit_scores(lc, qt, q0):
                    sl = bass.ts(lc, LC)
                    g_expf, g_probb = [], []
                    for g in range(NG):
                        sc_grp = attn.tile([S, GH, LC], F32, tag="sc_grp")
                        expf = attnL.tile([S, GH, LC], mybir.dt.float32r,
                                          tag="expf")
                        for j in range(GH):
                            h = g * GH + j
                            p0 = 64 * (h % 2)
                            cc = h // 2
                            sc_ps = ps_at.tile([S, LC], F32, tag="at")
                            nc.tensor.matmul(sc_ps, kt[p0:p0 + 64, cc, :],
                                             qt[p0:p0 + 64, cc, q0:q0 + LC],
                                             start=True, stop=True)
                            if j % 2 == 0:
                                nc.scalar.copy(sc_grp[:, j, :], sc_ps)
                            else:
                                nc.vector.tensor_copy(sc_grp[:, j, :], sc_ps)
                        nc.scalar.activation(expf, sc_grp,
                                             mybir.ActivationFunctionType.Exp)
                        nc.sync.dma_start(
                            out=d_scoreT.ap()[g * GH:(g + 1) * GH, :, sl]
                                .transpose([1, 0, 2]),
                            in_=sc_grp)
                        g_expf.append(expf)
                    for g in range(NG):
                        sums_ps = ps_at.tile([GH, LC], F32, tag="at")
                        for j in range(GH):
                            nc.tensor.matmul(sums_ps, eg[:, j, :],
                                             g_expf[g][:, j, :],
                                             start=(j == 0), stop=(j == GH - 1))
                        rec_sb = attn.tile([GH, LC], F32, tag="rec_sb")
                        nc.vector.reciprocal_approx_fast(rec_sb, sums_ps)
                        rec_dram = dramp.tile([GH, LC], F32, tag="rec_dram")
                        nc.sync.dma_start(out=rec_dram, in_=rec_sb)
                        rec_bg = attnL.tile([S, GH, LC], F32, tag="rec_bg")
                        rd = rec_dram[:, :]
                        nc.gpsimd.dma_start(
                            out=rec_bg,
                            in_=bass.AP(tensor=rd.tensor, offset=rd.offset,
                                        ap=[[0, S]] + list(rd.ap)))
                        probb = attnL.tile([S, GH, LC], BF16, tag="probb")
                        nc.vector.tensor_mul(probb, g_expf[g].bitcast(F32), rec_bg)
                        g_probb.append(probb)
                        probf = attn.tile([S, GH, LC], F32, tag="probf")
                        nc.vector.tensor_mul(probf, g_expf[g].bitcast(F32), rec_bg)
                        nc.sync.dma_start(
                            out=d_probT.ap()[g * GH:(g + 1) * GH, :, sl]
                                .transpose([1, 0, 2]),
                            in_=probf)
                    return g_probb

                def emit_attn_tail(lc, g_probb):
                    ot = outs.tile([128, KQ, LC], BF16, tag="ot")
                    # column-packed AV: even/odd head pair shares one PSUM
                    # tile via col tile_position -> concurrent in the array
                    for g in range(NG):
                        for j in range(0, GH, 2):
                            h = g * GH + j
                            cc = h // 2
                            av_ps = ps_at.tile([128, LC], F32, tag="at")
                            nc.tensor.matmul(av_ps[0:64, :], v_sb[:, h, :],
                                             g_probb[g][:, j, :],
                                             start=True, stop=True,
                                             tile_position=(0, 0))
                            nc.tensor.matmul(av_ps[64:128, :], v_sb[:, h + 1, :],
                                             g_probb[g][:, j + 1, :],
                                             start=True, stop=True,
                                             tile_position=(0, 64))
                            nc.any.tensor_copy(ot[:, cc, :], av_ps)
                    return ot

                def emit_outproj(lc, ot):
                    to_ps = ps_mm.tile([128, LC], F32, tag="ps_q")
                    for c in range(KQ):
                        nc.tensor.matmul(to_ps, a_o[:, c, :], ot[:, c, :],
                                         start=(c == 0), stop=(c == KQ - 1))
                    to_sb = outs.tile([128, LC], BF16, tag="to_sb")
                    nc.scalar.copy(to_sb, to_ps)
                    for m in range(LC // 128):
                        msl = bass.ts(m, 128)
                        for f0, fn in FO_CHUNKS:
                            ps_y = ps_mm.tile([128, 512], F32, tag="ps_y")
                            for c in range(KQ):
                                nc.tensor.matmul(ps_y[:, :fn], ot[:, c, msl],
                                                 w_o[:, c, f0:f0 + fn],
                                                 start=(c == 0), stop=False)
                            nc.tensor.matmul(ps_y[:, :fn], to_sb[:, msl],
                                             b_o[:, f0:f0 + fn],
                                             start=False, stop=True)
                            y_sb = outs.tile([128, 512], F32, tag="y_sb")
                            nc.vector.tensor_add(y_sb[:, :fn], ps_y[:, :fn],
                                                 bias_b[:, f0:f0 + fn])
                            r0 = lc * LC + m * 128
                            nc.sync.dma_start(
                                out=d_y.ap()[r0:r0 + 128, f0:f0 + fn],
                                in_=y_sb[:, :fn])

                # software-pipelined: q-proj of lc+1 is emitted between the
                # reciprocal round-trip of lc and its consumers, so the PE
                # always has independent matmuls to chew on
                qt = emit_qproj(0)
                for lc in range(NLC):
                    g_probb = emit_scores(lc, qt, 0)
                    if lc + 1 < NLC:
                        qt = emit_qproj(lc + 1)
                    ot = emit_attn_tail(lc, g_probb)
                    emit_outproj(lc, ot)

    nc.compile()
    return nc


def _get_program():
    global _COMPILED
    if _COMPILED is None:
        _COMPILED = _build_program()
    return _COMPILED


def _bf(x):
    return np.ascontiguousarray(np.asarray(x, dtype=np.float32)).astype(
        ml_dtypes.bfloat16)


def kernel(hidden_states, encoder_hidden_states, Wq, Wk, Wv, Wo, bo,
           Aq, Bq, Ak, Bk, Av, Bv, Ao, Bo):
    nc = _get_program()

    hidden_states = np.asarray(hidden_states, dtype=np.float32)
    encoder_hidden_states = np.asarray(encoder_hidden_states, dtype=np.float32)

    shared = dict(
        wqT=_bf(np.asarray(Wq).T.reshape(KQ, 128, INNER)),
        wkT=_bf((np.asarray(Wk) * SCORE_SCALE).T.reshape(KE, 128, INNER)),
        wvT=_bf(np.asarray(Wv).T.reshape(KE, 128, INNER)),
        woT=_bf(np.asarray(Wo).T.reshape(KQ, 128, INNER)),
        aqT=_bf(np.asarray(Aq).T.reshape(KQ, 128, LORA_R)),
        akT=_bf(np.asarray(Ak).T.reshape(KE, 128, LORA_R)),
        avT=_bf(np.asarray(Av).T.reshape(KE, 128, LORA_R)),
        aoT=_bf(np.asarray(Ao).T.reshape(KQ, 128, LORA_R)),
        bqT=_bf(LORA_SCALE * np.asarray(Bq).T),
        bkT=_bf(LORA_SCALE * SCORE_SCALE * np.asarray(Bk).T),
        bvT=_bf(LORA_SCALE * np.asarray(Bv).T),
        boT=_bf(LORA_SCALE * np.asarray(Bo).T),
        bias=np.ascontiguousarray(np.asarray(bo, dtype=np.float32)),
    )

    in_maps = []
    for i in range(B):
        in_maps.append(dict(
            xT=_bf(hidden_states[i].T).reshape(KQ, 128, L),
            encT=_bf(encoder_hidden_states[i].T).reshape(KE, 128, S),
            **shared))

    global LAST_RESULT
    res = run_bass_kernel_spmd(nc, in_maps, list(range(B)), trace=TRACE)
    LAST_RESULT = res

    out = np.empty((B, L, INNER), np.float32)
    score = np.empty((B, HEADS, L, S), np.float32)
    prob = np.empty((B, HEADS, L, S), np.float32)
    for i in range(B):
        r = res.results[i]
        out[i] = r["y"]
        score[i] = r["scoreT"].transpose(0, 2, 1)
        prob[i] = r["probT"].transpose(0, 2, 1)
    return out, score, prob


# revision 23
# speedup vs baseline: 1.0176x; 1.0176x over previous
"""Trainium2 Bass kernel for the AttnProcessor+LoRA capture problem.

Reference (per batch element):
    q = hs @ Wq.T + s*(hs @ Aq.T) @ Bq.T
    k = enc @ Wk.T + s*(enc @ Ak.T) @ Bk.T
    v = enc @ Wv.T + s*(enc @ Av.T) @ Bv.T
    score[h] = (q_h @ k_h.T) / 8 ;  prob = softmax(score, -1)
    out = prob @ v ;  y = out @ Wo.T + s*(out @ Ao.T) @ Bo.T + bo
Returns (y, score, prob).

Distribution: pure data-parallel over batch B=8 -> one element per
NeuronCore, no collectives.  Matmuls run bf16 with fp32 PSUM accumulation;
softmax in fp32.  Host-side layout prep: activations fed feature-major
([feat, tokens]) so contractions land on the partition axis; score/prob
produced as [H, S, L] on device (2KB-row DMAs) and permuted on host; the
1/sqrt(64) and LoRA scales are folded into the weights.  Softmax skips
max-subtraction (|score| < ~4) and normalizes via a gpsimd cross-partition
all-reduce of exp over S=77.
"""

import numpy as np
import ml_dtypes

import concourse.bass as bass
import concourse.tile as tile
from concourse import bacc, mybir
from concourse.bass_utils import run_bass_kernel_spmd

HEADS = 20
HEAD_DIM = 64
INNER = 1280
CTX = 768
LORA_R = 128
LORA_SCALE = 16.0 / 128.0
B, L, S = 8, 4096, 77
SCORE_SCALE = 1.0 / 8.0  # 1/sqrt(64)

LC = 256              # tokens per pipeline chunk
NLC = L // LC         # 16
KQ = INNER // 128     # 10
KE = CTX // 128       # 6
FQ = INNER // 128     # 10
GH = 4                # heads per attention group
NG = HEADS // GH      # 5
FO_CHUNKS = [(0, 512), (512, 512), (1024, 256)]

BF16 = mybir.dt.bfloat16
F32 = mybir.dt.float32

_COMPILED = None
TRACE = False          # set True (with axon_profile imported) to capture NTFF
LAST_RESULT = None     # BassKernelResults of the most recent kernel() call


def _build_program():
    nc = bacc.Bacc("TRN2", target_bir_lowering=False, debug=False)

    d_xT = nc.declare_dram_parameter("xT", [KQ, 128, L], BF16, isOutput=False)
    d_encT = nc.declare_dram_parameter("encT", [KE, 128, S], BF16, isOutput=False)
    d_wqT = nc.declare_dram_parameter("wqT", [KQ, 128, INNER], BF16, isOutput=False)
    d_wkT = nc.declare_dram_parameter("wkT", [KE, 128, INNER], BF16, isOutput=False)
    d_wvT = nc.declare_dram_parameter("wvT", [KE, 128, INNER], BF16, isOutput=False)
    d_woT = nc.declare_dram_parameter("woT", [KQ, 128, INNER], BF16, isOutput=False)
    d_aqT = nc.declare_dram_parameter("aqT", [KQ, 128, LORA_R], BF16, isOutput=False)
    d_akT = nc.declare_dram_parameter("akT", [KE, 128, LORA_R], BF16, isOutput=False)
    d_avT = nc.declare_dram_parameter("avT", [KE, 128, LORA_R], BF16, isOutput=False)
    d_aoT = nc.declare_dram_parameter("aoT", [KQ, 128, LORA_R], BF16, isOutput=False)
    d_bqT = nc.declare_dram_parameter("bqT", [LORA_R, INNER], BF16, isOutput=False)
    d_bkT = nc.declare_dram_parameter("bkT", [LORA_R, INNER], BF16, isOutput=False)
    d_bvT = nc.declare_dram_parameter("bvT", [LORA_R, INNER], BF16, isOutput=False)
    d_boT = nc.declare_dram_parameter("boT", [LORA_R, INNER], BF16, isOutput=False)
    d_bias = nc.declare_dram_parameter("bias", [INNER], F32, isOutput=False)

    d_y = nc.declare_dram_parameter("y", [L, INNER], F32, isOutput=True)
    d_scoreT = nc.declare_dram_parameter("scoreT", [HEADS, S, L], F32, isOutput=True)
    d_probT = nc.declare_dram_parameter("probT", [HEADS, S, L], F32, isOutput=True)

    with tile.TileContext(nc) as tc:
        with (
            tc.tile_pool(name="wts", bufs=1) as wts,
            tc.tile_pool(name="kv", bufs=1) as kvp,
        ):
            # ---- persistent weights ----
            w_q = wts.tile([128, KQ, INNER], BF16)
            w_o = wts.tile([128, KQ, INNER], BF16)
            a_q = wts.tile([128, KQ, LORA_R], BF16)
            a_o = wts.tile([128, KQ, LORA_R], BF16)
            b_q = wts.tile([128, INNER], BF16)
            b_o = wts.tile([128, INNER], BF16)
            bias_b = wts.tile([128, INNER], F32)
            for dst, src in ((w_q, d_wqT), (w_o, d_woT), (a_q, d_aqT), (a_o, d_aoT)):
                nc.sync.dma_start(out=dst, in_=src.ap().transpose([1, 0, 2]))
            nc.sync.dma_start(out=b_q, in_=d_bqT.ap())
            nc.sync.dma_start(out=b_o, in_=d_boT.ap())
            bias_ap = d_bias.ap()
            nc.gpsimd.dma_start(
                out=bias_b,
                in_=bass.AP(tensor=bias_ap.tensor, offset=bias_ap.offset,
                            ap=[[0, 128]] + list(bias_ap.ap)),
            )

            # k^T [feat, S] and v [S, heads, dim], computed once
            kt = kvp.tile([128, FQ, S], BF16)
            v_sb = kvp.tile([S, HEADS, HEAD_DIM], BF16)

            with (
                tc.tile_pool(name="setup", bufs=1) as stp,
                tc.tile_pool(name="ps_setup", bufs=2, space="PSUM") as pss,
            ):
                w_k = stp.tile([128, KE, INNER], BF16)
                w_v = stp.tile([128, KE, INNER], BF16)
                a_k = stp.tile([128, KE, LORA_R], BF16)
                a_v = stp.tile([128, KE, LORA_R], BF16)
                b_k = stp.tile([128, INNER], BF16)
                b_v = stp.tile([128, INNER], BF16)
                enc_sb = stp.tile([128, KE, S], BF16)
                for dst, src in ((w_k, d_wkT), (w_v, d_wvT),
                                 (a_k, d_akT), (a_v, d_avT)):
                    nc.sync.dma_start(out=dst, in_=src.ap().transpose([1, 0, 2]))
                nc.sync.dma_start(out=b_k, in_=d_bkT.ap())
                nc.sync.dma_start(out=b_v, in_=d_bvT.ap())
                nc.sync.dma_start(out=enc_sb, in_=d_encT.ap().transpose([1, 0, 2]))

                tk_ps = pss.tile([128, S], F32, tag="ps_lora")
                for c in range(KE):
                    nc.tensor.matmul(tk_ps, a_k[:, c, :], enc_sb[:, c, :],
                                     start=(c == 0), stop=(c == KE - 1))
                tk_sb = stp.tile([128, S], BF16)
                nc.scalar.copy(tk_sb, tk_ps)
                tv_ps = pss.tile([128, S], F32, tag="ps_lora")
                for c in range(KE):
                    nc.tensor.matmul(tv_ps, a_v[:, c, :], enc_sb[:, c, :],
                                     start=(c == 0), stop=(c == KE - 1))
                tv_sb = stp.tile([128, S], BF16)
                nc.scalar.copy(tv_sb, tv_ps)

                for f in range(FQ):
                    ps_k = pss.tile([128, S], F32, tag="ps_k")
                    for c in range(KE):
                        nc.tensor.matmul(ps_k, w_k[:, c, bass.ts(f, 128)],
                                         enc_sb[:, c, :], start=(c == 0), stop=False)
                    nc.tensor.matmul(ps_k, b_k[:, bass.ts(f, 128)], tk_sb,
                                     start=False, stop=True)
                    nc.scalar.copy(kt[:, f, :], ps_k)

                for f0, fn in FO_CHUNKS:
                    ps_v = pss.tile([S, 512], F32, tag="ps_v")
                    for c in range(KE):
                        nc.tensor.matmul(ps_v[:, :fn], enc_sb[:, c, :],
                                         w_v[:, c, f0:f0 + fn],
                                         start=(c == 0), stop=False)
                    nc.tensor.matmul(ps_v[:, :fn], tv_sb, b_v[:, f0:f0 + fn],
                                     start=False, stop=True)
                    nc.vector.tensor_copy(
                        v_sb[:, f0 // HEAD_DIM:(f0 + fn) // HEAD_DIM, :],
                        ps_v[:, :fn].rearrange("s (h d) -> s h d", d=HEAD_DIM),
                    )

            # ---- main pipeline over L-chunks ----
            with (
                tc.tile_pool(name="acts", bufs=2) as acts,
                tc.tile_pool(name="attn", bufs=3) as attn,
                tc.tile_pool(name="attnL", bufs=NG + 2) as attnL,
                tc.tile_pool(name="outs", bufs=2) as outs,
                tc.tile_pool(name="ps_mm", bufs=2, space="PSUM") as ps_mm,
                tc.tile_pool(name="ps_at", bufs=4, space="PSUM") as ps_at,
                tc.tile_pool(name="dram", bufs=NG + 1, space="DRAM") as dramp,
            ):
                # indicator matrices: eg[:, j, :] is [S, GH] with column j
                # all-ones -> matmul accumulates head j's column-sums into
                # row j of a [GH, LC] PSUM tile
                eg_f = wts.tile([S, GH, GH], F32)
                nc.vector.memset(eg_f, 0.0)
                for j in range(GH):
                    nc.vector.memset(eg_f[:, j, j:j + 1], 1.0)
                eg = wts.tile([S, GH, GH], mybir.dt.float32r)
                nc.vector.tensor_copy(eg, eg_f)
                def emit_qproj(lc):
                    sl = bass.ts(lc, LC)
                    xq = acts.tile([128, KQ, LC], BF16, tag="xq")
                    nc.sync.dma_start(out=xq,
                                      in_=d_xT.ap()[:, :, sl].transpose([1, 0, 2]))
                    tq_ps = ps_mm.tile([128, LC], F32, tag="ps_q")
                    for c in range(KQ):
                        nc.tensor.matmul(tq_ps, a_q[:, c, :], xq[:, c, :],
                                         start=(c == 0), stop=(c == KQ - 1))
                    tq = acts.tile([128, LC], BF16, tag="tq")
                    nc.scalar.copy(tq, tq_ps)
                    qt = acts.tile([128, FQ, LC], BF16, tag="qt")
                    for f in range(FQ):
                        ps_q = ps_mm.tile([128, LC], F32, tag="ps_q")
                        for c in range(KQ):
                            nc.tensor.matmul(ps_q, w_q[:, c, bass.ts(f, 128)],
                                             xq[:, c, :], start=(c == 0), stop=False)
                        nc.tensor.matmul(ps_q, b_q[:, bass.ts(f, 128)], tq,
                                         start=False, stop=True)
                        nc.scalar.copy(qt[:, f, :], ps_q)
                    return qt

                def emit_scores(lc, qt, q0):
                    sl = bass.ts(lc, LC)
                    g_expf, g_probb = [], []
                    for g in range(NG):
                        sc_grp = attn.tile([S, GH, LC], F32, tag="sc_grp")
                        expf = attnL.tile([S, GH, LC], mybir.dt.float32r,
                                          tag="expf")
                        for j in range(GH):
                            h = g * GH + j
                            p0 = 64 * (h % 2)
                            cc = h // 2
                            sc_ps = ps_at.tile([S, LC], F32, tag="at")
                            nc.tensor.matmul(sc_ps, kt[p0:p0 + 64, cc, :],
                                             qt[p0:p0 + 64, cc, q0:q0 + LC],
                                             start=True, stop=True)
                            nc.any.tensor_copy(sc_grp[:, j, :], sc_ps)
                        nc.scalar.activation(expf, sc_grp,
                                             mybir.ActivationFunctionType.Exp)
                        nc.sync.dma_start(
                            out=d_scoreT.ap()[g * GH:(g + 1) * GH, :, sl]
                                .transpose([1, 0, 2]),
                            in_=sc_grp)
                        g_expf.append(expf)
                    for g in range(NG):
                        sums_ps = ps_at.tile([GH, LC], F32, tag="at")
                        for j in range(GH):
                            nc.tensor.matmul(sums_ps, eg[:, j, :],
                                             g_expf[g][:, j, :],
                                             start=(j == 0), stop=(j == GH - 1))
                        rec_sb = attn.tile([GH, LC], F32, tag="rec_sb")
                        nc.vector.reciprocal_approx_fast(rec_sb, sums_ps)
                        rec_dram = dramp.tile([GH, LC], F32, tag="rec_dram")
                        nc.sync.dma_start(out=rec_dram, in_=rec_sb)
                        rec_bg = attnL.tile([S, GH, LC], F32, tag="rec_bg")
                        rd = rec_dram[:, :]
                        nc.gpsimd.dma_start(
                            out=rec_bg,
                            in_=bass.AP(tensor=rd.tensor, offset=rd.offset,
                                        ap=[[0, S]] + list(rd.ap)))
                        probb = attnL.tile([S, GH, LC], BF16, tag="probb")
                        nc.vector.tensor_mul(probb, g_expf[g].bitcast(F32), rec_bg)
                        g_probb.append(probb)
                        probf = attn.tile([S, GH, LC], F32, tag="probf")
                        nc.vector.tensor_mul(probf, g_expf[g].bitcast(F32), rec_bg)
                        nc.sync.dma_start(
                            out=d_probT.ap()[g * GH:(g + 1) * GH, :, sl]
                                .transpose([1, 0, 2]),
                            in_=probf)
                    return g_probb

                def emit_attn_tail(lc, g_probb):
                    ot = outs.tile([128, KQ, LC], BF16, tag="ot")
                    # column-packed AV: even/odd head pair shares one PSUM
                    # tile via col tile_position -> concurrent in the array
                    for g in range(NG):
                        for j in range(0, GH, 2):
                            h = g * GH + j
                            cc = h // 2
                            av_ps = ps_at.tile([128, LC], F32, tag="at")
                            nc.tensor.matmul(av_ps[0:64, :], v_sb[:, h, :],
                                             g_probb[g][:, j, :],
                                             start=True, stop=True,
                                             tile_position=(0, 0))
                            nc.tensor.matmul(av_ps[64:128, :], v_sb[:, h + 1, :],
                                             g_probb[g][:, j + 1, :],
                                             start=True, stop=True,
                                             tile_position=(0, 64))
                            nc.any.tensor_copy(ot[:, cc, :], av_ps)
                    return ot

                def emit_outproj(lc, ot):
                    to_ps = ps_mm.tile([128, LC], F32, tag="ps_q")
                    for c in range(KQ):
                        nc.tensor.matmul(to_ps, a_o[:, c, :], ot[:, c, :],
                                         start=(c == 0), stop=(c == KQ - 1))
                    to_sb = outs.tile([128, LC], BF16, tag="to_sb")
                    nc.scalar.copy(to_sb, to_ps)
                    for m in range(LC // 128):
                        msl = bass.ts(m, 128)
                        for f0, fn in FO_CHUNKS:
                            ps_y = ps_mm.tile([128, 512], F32, tag="ps_y")
                            for c in range(KQ):
                                nc.tensor.matmul(ps_y[:, :fn], ot[:, c, msl],
                                                 w_o[:, c, f0:f0 + fn],
                                                 start=(c == 0), stop=False)
                            nc.tensor.matmul(ps_y[:, :fn], to_sb[:, msl],
                                             b_o[:, f0:f0 + fn],
                                             start=False, stop=True)
                            y_sb = outs.tile([128, 512], F32, tag="y_sb")
                            nc.vector.tensor_add(y_sb[:, :fn], ps_y[:, :fn],
                                                 bias_b[:, f0:f0 + fn])
                            r0 = lc * LC + m * 128
                            nc.sync.dma_start(
                                out=d_y.ap()[r0:r0 + 128, f0:f0 + fn],
                                in_=y_sb[:, :fn])

                # software-pipelined: q-proj of lc+1 is emitted between the
                # reciprocal round-trip of lc and its consumers, so the PE
                # always has independent matmuls to chew on
                qt = emit_qproj(0)
                for lc in range(NLC):
                    g_probb = emit_scores(lc, qt, 0)
                    if lc + 1 < NLC:
                        qt = emit_qproj(lc + 1)
                    ot = emit_attn_tail(lc, g_probb)
                    emit_outproj(lc, ot)

    nc.compile()
    return nc


def _get_program():
    global _COMPILED
    if _COMPILED is None:
        _COMPILED = _build_program()
    return _COMPILED


def _bf(x):
    return np.ascontiguousarray(np.asarray(x, dtype=np.float32)).astype(
        ml_dtypes.bfloat16)


def kernel(hidden_states, encoder_hidden_states, Wq, Wk, Wv, Wo, bo,
           Aq, Bq, Ak, Bk, Av, Bv, Ao, Bo):
    nc = _get_program()

    hidden_states = np.asarray(hidden_states, dtype=np.float32)
    encoder_hidden_states = np.asarray(encoder_hidden_states, dtype=np.float32)

    shared = dict(
        wqT=_bf(np.asarray(Wq).T.reshape(KQ, 128, INNER)),
        wkT=_bf((np.asarray(Wk) * SCORE_SCALE).T.reshape(KE, 128, INNER)),
        wvT=_bf(np.asarray(Wv).T.reshape(KE, 128, INNER)),
        woT=_bf(np.asarray(Wo).T.reshape(KQ, 128, INNER)),
        aqT=_bf(np.asarray(Aq).T.reshape(KQ, 128, LORA_R)),
        akT=_bf(np.asarray(Ak).T.reshape(KE, 128, LORA_R)),
        avT=_bf(np.asarray(Av).T.reshape(KE, 128, LORA_R)),
        aoT=_bf(np.asarray(Ao).T.reshape(KQ, 128, LORA_R)),
        bqT=_bf(LORA_SCALE * np.asarray(Bq).T),
        bkT=_bf(LORA_SCALE * SCORE_SCALE * np.asarray(Bk).T),
        bvT=_bf(LORA_SCALE * np.asarray(Bv).T),
        boT=_bf(LORA_SCALE * np.asarray(Bo).T),
        bias=np.ascontiguousarray(np.asarray(bo, dtype=np.float32)),
    )

    in_maps = []
    for i in range(B):
        in_maps.append(dict(
            xT=_bf(hidden_states[i].T).reshape(KQ, 128, L),
            encT=_bf(encoder_hidden_states[i].T).reshape(KE, 128, S),
            **shared))

    global LAST_RESULT
    res = run_bass_kernel_spmd(nc, in_maps, list(range(B)), trace=TRACE)
    LAST_RESULT = res

    out = np.empty((B, L, INNER), np.float32)
    score = np.empty((B, HEADS, L, S), np.float32)
    prob = np.empty((B, HEADS, L, S), np.float32)
    for i in range(B):
        r = res.results[i]
        out[i] = r["y"]
        score[i] = r["scoreT"].transpose(0, 2, 1)
        prob[i] = r["probT"].transpose(0, 2, 1)
    return out, score, prob


# revision 24
# speedup vs baseline: 1.0389x; 1.0210x over previous
"""Trainium2 Bass kernel for the AttnProcessor+LoRA capture problem.

Reference (per batch element):
    q = hs @ Wq.T + s*(hs @ Aq.T) @ Bq.T
    k = enc @ Wk.T + s*(enc @ Ak.T) @ Bk.T
    v = enc @ Wv.T + s*(enc @ Av.T) @ Bv.T
    score[h] = (q_h @ k_h.T) / 8 ;  prob = softmax(score, -1)
    out = prob @ v ;  y = out @ Wo.T + s*(out @ Ao.T) @ Bo.T + bo
Returns (y, score, prob).

Distribution: pure data-parallel over batch B=8 -> one element per
NeuronCore, no collectives.  Matmuls run bf16 with fp32 PSUM accumulation;
softmax in fp32.  Host-side layout prep: activations fed feature-major
([feat, tokens]) so contractions land on the partition axis; score/prob
produced as [H, S, L] on device (2KB-row DMAs) and permuted on host; the
1/sqrt(64) and LoRA scales are folded into the weights.  Softmax skips
max-subtraction (|score| < ~4); the per-(head, token) exp-sums are built by
tiny indicator matmuls on the TensorEngine, inverted with the fast DVE
reciprocal, and broadcast across the S=77 partitions by a stride-0 DMA
read-back from a DRAM scratch tile.  The main loop is software-pipelined:
chunk lc+1's q-projection matmuls are emitted between chunk lc's reciprocal
round-trip and its consumers so the TensorEngine never starves.
"""

import sys

for _p in ("/root/.axon_site/_ro/trn_rl_repo", "/opt/trn_rl_repo"):
    if _p not in sys.path:
        sys.path.append(_p)

import numpy as np
import ml_dtypes

import concourse.bass as bass
import concourse.tile as tile
from concourse import bacc, mybir
from concourse.bass_utils import run_bass_kernel_spmd

HEADS = 20
HEAD_DIM = 64
INNER = 1280
CTX = 768
LORA_R = 128
LORA_SCALE = 16.0 / 128.0
B, L, S = 8, 4096, 77
SCORE_SCALE = 1.0 / 8.0  # 1/sqrt(64)

LC = 256              # tokens per pipeline chunk
NLC = L // LC         # 16
KQ = INNER // 128     # 10
KE = CTX // 128       # 6
FQ = INNER // 128     # 10
GH = 4                # heads per attention group
NG = HEADS // GH      # 5
FO_CHUNKS = [(0, 512), (512, 512), (1024, 256)]

BF16 = mybir.dt.bfloat16
F32 = mybir.dt.float32

_COMPILED = None
TRACE = False          # set True (with axon_profile imported) to capture NTFF
LAST_RESULT = None     # BassKernelResults of the most recent kernel() call


def _build_program():
    nc = bacc.Bacc("TRN2", target_bir_lowering=False, debug=False)

    d_xT = nc.declare_dram_parameter("xT", [KQ, 128, L], BF16, isOutput=False)
    d_encT = nc.declare_dram_parameter("encT", [KE, 128, S], BF16, isOutput=False)
    d_wqT = nc.declare_dram_parameter("wqT", [KQ, 128, INNER], BF16, isOutput=False)
    d_wkT = nc.declare_dram_parameter("wkT", [KE, 128, INNER], BF16, isOutput=False)
    d_wvT = nc.declare_dram_parameter("wvT", [KE, 128, INNER], BF16, isOutput=False)
    d_woT = nc.declare_dram_parameter("woT", [KQ, 128, INNER], BF16, isOutput=False)
    d_aqT = nc.declare_dram_parameter("aqT", [KQ, 128, LORA_R], BF16, isOutput=False)
    d_akT = nc.declare_dram_parameter("akT", [KE, 128, LORA_R], BF16, isOutput=False)
    d_avT = nc.declare_dram_parameter("avT", [KE, 128, LORA_R], BF16, isOutput=False)
    d_aoT = nc.declare_dram_parameter("aoT", [KQ, 128, LORA_R], BF16, isOutput=False)
    d_bqT = nc.declare_dram_parameter("bqT", [LORA_R, INNER], BF16, isOutput=False)
    d_bkT = nc.declare_dram_parameter("bkT", [LORA_R, INNER], BF16, isOutput=False)
    d_bvT = nc.declare_dram_parameter("bvT", [LORA_R, INNER], BF16, isOutput=False)
    d_boT = nc.declare_dram_parameter("boT", [LORA_R, INNER], BF16, isOutput=False)
    d_bias = nc.declare_dram_parameter("bias", [INNER], F32, isOutput=False)

    d_y = nc.declare_dram_parameter("y", [L, INNER], F32, isOutput=True)
    d_scoreT = nc.declare_dram_parameter("scoreT", [HEADS, S, L], F32, isOutput=True)
    d_probT = nc.declare_dram_parameter("probT", [HEADS, S, L], F32, isOutput=True)

    with tile.TileContext(nc) as tc:
        with (
            tc.tile_pool(name="wts", bufs=1) as wts,
            tc.tile_pool(name="kv", bufs=1) as kvp,
        ):
            # ---- persistent weights ----
            w_q = wts.tile([128, KQ, INNER], BF16)
            w_o = wts.tile([128, KQ, INNER], BF16)
            a_q = wts.tile([128, KQ, LORA_R], BF16)
            a_o = wts.tile([128, KQ, LORA_R], BF16)
            b_q = wts.tile([128, INNER], BF16)
            b_o = wts.tile([128, INNER], BF16)
            bias_b = wts.tile([128, INNER], F32)
            for dst, src in ((w_q, d_wqT), (w_o, d_woT), (a_q, d_aqT), (a_o, d_aoT)):
                nc.sync.dma_start(out=dst, in_=src.ap().transpose([1, 0, 2]))
            nc.sync.dma_start(out=b_q, in_=d_bqT.ap())
            nc.sync.dma_start(out=b_o, in_=d_boT.ap())
            bias_ap = d_bias.ap()
            nc.gpsimd.dma_start(
                out=bias_b,
                in_=bass.AP(tensor=bias_ap.tensor, offset=bias_ap.offset,
                            ap=[[0, 128]] + list(bias_ap.ap)),
            )

            # k^T [feat, S] and v [S, heads, dim], computed once
            kt = kvp.tile([128, FQ, S], BF16)
            v_sb = kvp.tile([S, HEADS, HEAD_DIM], BF16)

            with (
                tc.tile_pool(name="setup", bufs=1) as stp,
                tc.tile_pool(name="ps_setup", bufs=2, space="PSUM") as pss,
            ):
                w_k = stp.tile([128, KE, INNER], BF16)
                w_v = stp.tile([128, KE, INNER], BF16)
                a_k = stp.tile([128, KE, LORA_R], BF16)
                a_v = stp.tile([128, KE, LORA_R], BF16)
                b_k = stp.tile([128, INNER], BF16)
                b_v = stp.tile([128, INNER], BF16)
                enc_sb = stp.tile([128, KE, S], BF16)
                for dst, src in ((w_k, d_wkT), (w_v, d_wvT),
                                 (a_k, d_akT), (a_v, d_avT)):
                    nc.sync.dma_start(out=dst, in_=src.ap().transpose([1, 0, 2]))
                nc.sync.dma_start(out=b_k, in_=d_bkT.ap())
                nc.sync.dma_start(out=b_v, in_=d_bvT.ap())
                nc.sync.dma_start(out=enc_sb, in_=d_encT.ap().transpose([1, 0, 2]))

                tk_ps = pss.tile([128, S], F32, tag="ps_lora")
                for c in range(KE):
                    nc.tensor.matmul(tk_ps, a_k[:, c, :], enc_sb[:, c, :],
                                     start=(c == 0), stop=(c == KE - 1))
                tk_sb = stp.tile([128, S], BF16)
                nc.scalar.copy(tk_sb, tk_ps)
                tv_ps = pss.tile([128, S], F32, tag="ps_lora")
                for c in range(KE):
                    nc.tensor.matmul(tv_ps, a_v[:, c, :], enc_sb[:, c, :],
                                     start=(c == 0), stop=(c == KE - 1))
                tv_sb = stp.tile([128, S], BF16)
                nc.scalar.copy(tv_sb, tv_ps)

                for f in range(FQ):
                    ps_k = pss.tile([128, S], F32, tag="ps_k")
                    for c in range(KE):
                        nc.tensor.matmul(ps_k, w_k[:, c, bass.ts(f, 128)],
                                         enc_sb[:, c, :], start=(c == 0), stop=False)
                    nc.tensor.matmul(ps_k, b_k[:, bass.ts(f, 128)], tk_sb,
                                     start=False, stop=True)
                    nc.scalar.copy(kt[:, f, :], ps_k)

                for f0, fn in FO_CHUNKS:
                    ps_v = pss.tile([S, 512], F32, tag="ps_v")
                    for c in range(KE):
                        nc.tensor.matmul(ps_v[:, :fn], enc_sb[:, c, :],
                                         w_v[:, c, f0:f0 + fn],
                                         start=(c == 0), stop=False)
                    nc.tensor.matmul(ps_v[:, :fn], tv_sb, b_v[:, f0:f0 + fn],
                                     start=False, stop=True)
                    nc.vector.tensor_copy(
                        v_sb[:, f0 // HEAD_DIM:(f0 + fn) // HEAD_DIM, :],
                        ps_v[:, :fn].rearrange("s (h d) -> s h d", d=HEAD_DIM),
                    )

            # ---- main pipeline over L-chunks ----
            with (
                tc.tile_pool(name="acts", bufs=2) as acts,
                tc.tile_pool(name="attn", bufs=3) as attn,
                tc.tile_pool(name="attnL", bufs=NG + 1) as attnL,
                tc.tile_pool(name="outs", bufs=2) as outs,
                tc.tile_pool(name="ps_mm", bufs=2, space="PSUM") as ps_mm,
                tc.tile_pool(name="ps_at", bufs=4, space="PSUM") as ps_at,
                tc.tile_pool(name="dram", bufs=NG + 1, space="DRAM") as dramp,
            ):
                # indicator matrices: eg[:, j, :] is [S, GH] with column j
                # all-ones -> matmul accumulates head j's column-sums into
                # row j of a [GH, LC] PSUM tile
                eg_f = wts.tile([S, GH, GH], F32)
                nc.vector.memset(eg_f, 0.0)
                for j in range(GH):
                    nc.vector.memset(eg_f[:, j, j:j + 1], 1.0)
                eg = wts.tile([S, GH, GH], mybir.dt.float32r)
                nc.vector.tensor_copy(eg, eg_f)
                def emit_qproj(lc):
                    sl = bass.ts(lc, LC)
                    xq = acts.tile([128, KQ, LC], BF16, tag="xq")
                    nc.sync.dma_start(out=xq,
                                      in_=d_xT.ap()[:, :, sl].transpose([1, 0, 2]))
                    tq_ps = ps_mm.tile([128, LC], F32, tag="ps_q")
                    for c in range(KQ):
                        nc.tensor.matmul(tq_ps, a_q[:, c, :], xq[:, c, :],
                                         start=(c == 0), stop=(c == KQ - 1))
                    tq = acts.tile([128, LC], BF16, tag="tq")
                    nc.scalar.copy(tq, tq_ps)
                    qt = acts.tile([128, FQ, LC], BF16, tag="qt")
                    for f in range(FQ):
                        ps_q = ps_mm.tile([128, LC], F32, tag="ps_q")
                        for c in range(KQ):
                            nc.tensor.matmul(ps_q, w_q[:, c, bass.ts(f, 128)],
                                             xq[:, c, :], start=(c == 0), stop=False)
                        nc.tensor.matmul(ps_q, b_q[:, bass.ts(f, 128)], tq,
                                         start=False, stop=True)
                        nc.scalar.copy(qt[:, f, :], ps_q)
                    return qt

                def emit_scores(lc, qt, q0):
                    sl = bass.ts(lc, LC)
                    g_expf, g_probb = [], []
                    for g in range(NG):
                        sc_grp = attn.tile([S, GH, LC], F32, tag="sc_grp")
                        expf = attnL.tile([S, GH, LC], mybir.dt.float32r,
                                          tag="expf")
                        for j in range(GH):
                            h = g * GH + j
                            p0 = 64 * (h % 2)
                            cc = h // 2
                            sc_ps = ps_at.tile([S, LC], F32, tag="at")
                            nc.tensor.matmul(sc_ps, kt[p0:p0 + 64, cc, :],
                                             qt[p0:p0 + 64, cc, q0:q0 + LC],
                                             start=True, stop=True)
                            nc.any.tensor_copy(sc_grp[:, j, :], sc_ps)
                        nc.scalar.activation(expf, sc_grp,
                                             mybir.ActivationFunctionType.Exp)
                        nc.sync.dma_start(
                            out=d_scoreT.ap()[g * GH:(g + 1) * GH, :, sl]
                                .transpose([1, 0, 2]),
                            in_=sc_grp)
                        g_expf.append(expf)
                    for g in range(NG):
                        sums_ps = ps_at.tile([GH, LC], F32, tag="at")
                        for j in range(GH):
                            nc.tensor.matmul(sums_ps, eg[:, j, :],
                                             g_expf[g][:, j, :],
                                             start=(j == 0), stop=(j == GH - 1))
                        rec_sb = attn.tile([GH, LC], F32, tag="rec_sb")
                        nc.vector.reciprocal_approx_fast(rec_sb, sums_ps)
                        rec_dram = dramp.tile([GH, LC], F32, tag="rec_dram")
                        nc.sync.dma_start(out=rec_dram, in_=rec_sb)
                        rec_bg = attnL.tile([S, GH, LC], F32, tag="rec_bg")
                        rd = rec_dram[:, :]
                        nc.gpsimd.dma_start(
                            out=rec_bg,
                            in_=bass.AP(tensor=rd.tensor, offset=rd.offset,
                                        ap=[[0, S]] + list(rd.ap)))
                        probb = attnL.tile([S, GH, LC], BF16, tag="probb")
                        nc.vector.tensor_mul(probb, g_expf[g].bitcast(F32), rec_bg)
                        g_probb.append(probb)
                        probf = attn.tile([S, GH, LC], F32, tag="probf")
                        nc.vector.tensor_mul(probf, g_expf[g].bitcast(F32), rec_bg)
                        nc.sync.dma_start(
                            out=d_probT.ap()[g * GH:(g + 1) * GH, :, sl]
                                .transpose([1, 0, 2]),
                            in_=probf)
                    return g_probb

                def emit_attn_tail(lc, g_probb):
                    ot = outs.tile([128, KQ, LC], BF16, tag="ot")
                    # column-packed AV: even/odd head pair shares one PSUM
                    # tile via col tile_position -> concurrent in the array
                    for g in range(NG):
                        for j in range(0, GH, 2):
                            h = g * GH + j
                            cc = h // 2
                            av_ps = ps_at.tile([128, LC], F32, tag="at")
                            nc.tensor.matmul(av_ps[0:64, :], v_sb[:, h, :],
                                             g_probb[g][:, j, :],
                                             start=True, stop=True,
                                             tile_position=(0, 0))
                            nc.tensor.matmul(av_ps[64:128, :], v_sb[:, h + 1, :],
                                             g_probb[g][:, j + 1, :],
                                             start=True, stop=True,
                                             tile_position=(0, 64))
                            nc.any.tensor_copy(ot[:, cc, :], av_ps)
                    return ot

                def emit_outproj(lc, ot):
                    to_ps = ps_mm.tile([128, LC], F32, tag="ps_q")
                    for c in range(KQ):
                        nc.tensor.matmul(to_ps, a_o[:, c, :], ot[:, c, :],
                                         start=(c == 0), stop=(c == KQ - 1))
                    to_sb = outs.tile([128, LC], BF16, tag="to_sb")
                    nc.scalar.copy(to_sb, to_ps)
                    for m in range(LC // 128):
                        msl = bass.ts(m, 128)
                        for f0, fn in FO_CHUNKS:
                            ps_y = ps_mm.tile([128, 512], F32, tag="ps_y")
                            for c in range(KQ):
                                nc.tensor.matmul(ps_y[:, :fn], ot[:, c, msl],
                                                 w_o[:, c, f0:f0 + fn],
                                                 start=(c == 0), stop=False)
                            nc.tensor.matmul(ps_y[:, :fn], to_sb[:, msl],
                                             b_o[:, f0:f0 + fn],
                                             start=False, stop=True)
                            y_sb = outs.tile([128, 512], F32, tag="y_sb")
                            nc.vector.tensor_add(y_sb[:, :fn], ps_y[:, :fn],
                                                 bias_b[:, f0:f0 + fn])
                            r0 = lc * LC + m * 128
                            nc.sync.dma_start(
                                out=d_y.ap()[r0:r0 + 128, f0:f0 + fn],
                                in_=y_sb[:, :fn])

                # software-pipelined: q-proj of lc+1 is emitted between the
                # reciprocal round-trip of lc and its consumers, so the PE
                # always has independent matmuls to chew on
                qt = emit_qproj(0)
                for lc in range(NLC):
                    g_probb = emit_scores(lc, qt, 0)
                    if lc + 1 < NLC:
                        qt = emit_qproj(lc + 1)
                    ot = emit_attn_tail(lc, g_probb)
                    emit_outproj(lc, ot)

    nc.compile()
    return nc


def _get_program():
    global _COMPILED
    if _COMPILED is None:
        _COMPILED = _build_program()
    return _COMPILED


def _bf(x):
    return np.ascontiguousarray(np.asarray(x, dtype=np.float32)).astype(
        ml_dtypes.bfloat16)


def kernel(hidden_states, encoder_hidden_states, Wq, Wk, Wv, Wo, bo,
           Aq, Bq, Ak, Bk, Av, Bv, Ao, Bo):
    nc = _get_program()

    hidden_states = np.asarray(hidden_states, dtype=np.float32)
    encoder_hidden_states = np.asarray(encoder_hidden_states, dtype=np.float32)

    shared = dict(
        wqT=_bf(np.asarray(Wq).T.reshape(KQ, 128, INNER)),
        wkT=_bf((np.asarray(Wk) * SCORE_SCALE).T.reshape(KE, 128, INNER)),
        wvT=_bf(np.asarray(Wv).T.reshape(KE, 128, INNER)),
        woT=_bf(np.asarray(Wo).T.reshape(KQ, 128, INNER)),
        aqT=_bf(np.asarray(Aq).T.reshape(KQ, 128, LORA_R)),
        akT=_bf(np.asarray(Ak).T.reshape(KE, 128, LORA_R)),
        avT=_bf(np.asarray(Av).T.reshape(KE, 128, LORA_R)),
        aoT=_bf(np.asarray(Ao).T.reshape(KQ, 128, LORA_R)),
        bqT=_bf(LORA_SCALE * np.asarray(Bq).T),
        bkT=_bf(LORA_SCALE * SCORE_SCALE * np.asarray(Bk).T),
        bvT=_bf(LORA_SCALE * np.asarray(Bv).T),
        boT=_bf(LORA_SCALE * np.asarray(Bo).T),
        bias=np.ascontiguousarray(np.asarray(bo, dtype=np.float32)),
    )

    in_maps = []
    for i in range(B):
        in_maps.append(dict(
            xT=_bf(hidden_states[i].T).reshape(KQ, 128, L),
            encT=_bf(encoder_hidden_states[i].T).reshape(KE, 128, S),
            **shared))

    global LAST_RESULT
    res = run_bass_kernel_spmd(nc, in_maps, list(range(B)), trace=TRACE)
    LAST_RESULT = res

    out = np.empty((B, L, INNER), np.float32)
    score = np.empty((B, HEADS, L, S), np.float32)
    prob = np.empty((B, HEADS, L, S), np.float32)
    for i in range(B):
        r = res.results[i]
        out[i] = r["y"]
        score[i] = r["scoreT"].transpose(0, 2, 1)
        prob[i] = r["probT"].transpose(0, 2, 1)
    return out, score, prob
